# revision 1
# baseline (speedup 1.0000x reference)
"""TextLSTM kernel for 8 Trainium2 NeuronCores.

Data-parallel over batch: each of the 8 cores runs the full model on a
512-row batch shard.

Per-core pipeline (feature-major LSTM):
  1. Embedding gather: indirect-DMA 2560 rows of Emb (t-major token order)
     into SBUF batch-major, PE-transpose 128x128 blocks into feature-major
     xT[t] tiles (cast bf16).
  2. LSTM recurrence, 5 steps: gates[4H, 512b] = WT.T @ [h; x_t] computed as
     128x128 (bf16 weight stationary) x [128, 512] (bf16 h/x moving) matmuls
     accumulated in fp32 PSUM; sigmoid/tanh on ScalarE (gate bias folded in);
     cell math fp32 on VectorE; h stored bf16 (double-buffered), c fp32.
  3. Projection: out[512b, 32000v] = h.T @ WoutT streamed per 512-col vocab
     tile (bf16 weights, fp32 PSUM/output).

Weights are pre-transposed/tiled/cast on the host; biases are all zero per
the problem spec (gate biases are still applied on-device via the activation
bias port; bout is added on host only if nonzero).
"""

import os
import sys

import numpy as np
import ml_dtypes

for _p in ("/opt/trn_rl_repo", "/root/.axon_site/_ro/trn_rl_repo"):
    if os.path.isdir(_p) and _p not in sys.path:
        sys.path.append(_p)

from concourse import bacc, mybir
import concourse.tile as tile
from concourse.bass import IndirectOffsetOnAxis
from concourse.bass_utils import run_bass_kernel_spmd
from concourse.masks import make_identity

P = 128
B, T, E, H, V = 4096, 5, 512, 1024, 32000
NCORES = 8
BS = B // NCORES          # 512 batch rows per core
NTOK = BS * T             # 2560 gathered tokens per core
NG = NTOK // P            # 20 gather tiles of 128 tokens
KH = H // P               # 8 k-tiles over h
KE = E // P               # 4 k-tiles over x
KHX = KH + KE             # 12 k-tiles over [h; x]
NJ = H // P               # 8 hidden-dim tiles
VN = 512                  # vocab tile width
VT = (V + VN - 1) // VN   # 63 vocab tiles (last one 256 wide)
VPAD = VT * VN            # 32256
NBT = BS // P             # 4 batch tiles

F32 = mybir.dt.float32
BF16 = mybir.dt.bfloat16
I32 = mybir.dt.int32
AF = mybir.ActivationFunctionType

_BF = ml_dtypes.bfloat16

_CACHE = {}
LAST_RESULTS = None


def _build():
    nc = bacc.Bacc("TRN2", target_bir_lowering=False, debug=False,
                   num_devices=NCORES)

    idx_d = nc.dram_tensor("idx", [P, NG], I32, kind="ExternalInput")
    emb_d = nc.dram_tensor("emb", [V, E], BF16, kind="ExternalInput")
    wt_d = nc.dram_tensor("wt", [P, KHX, 4 * H], BF16, kind="ExternalInput")
    bias_d = nc.dram_tensor("bias", [P, 4 * H // P], F32, kind="ExternalInput")
    wo_d = nc.dram_tensor("wo", [VT, P, KH * VN], BF16, kind="ExternalInput")
    out_d = nc.dram_tensor("out", [BS, V], F32, kind="ExternalOutput")

    with tile.TileContext(nc) as tc:
        with (
            tc.tile_pool(name="const", bufs=1) as cpool,
            tc.tile_pool(name="gather", bufs=6) as gpool,
            tc.tile_pool(name="work", bufs=2) as wpool,
            tc.tile_pool(name="woutp", bufs=3) as wopool,
            tc.tile_pool(name="outp", bufs=4) as opool,
            tc.tile_pool(name="psum", bufs=8, space="PSUM") as pspool,
        ):
            ident = cpool.tile([P, P], BF16, tag="ident")
            make_identity(nc, ident[:])

            # persistent SBUF state
            wt_sb = cpool.tile([P, KHX, 4 * H], BF16, tag="wt")
            bias_sb = cpool.tile([P, 4 * H // P], F32, tag="bias")
            h_sb = cpool.tile([P, 2, KH, BS], BF16, tag="h")
            c_sb = cpool.tile([P, NJ, BS], F32, tag="c")
            xt_sb = cpool.tile([P, T, KE, BS], BF16, tag="xt")
            idx_sb = cpool.tile([P, NG], I32, tag="idx")

            nc.sync.dma_start(out=idx_sb[:], in_=idx_d.ap())
            nc.sync.dma_start(out=bias_sb[:], in_=bias_d.ap())
            # x-part weights (k 8..11) first: they gate the t=0 matmuls,
            # the h-part loads overlap with t=0 compute.
            for kt in list(range(KH, KHX)) + list(range(KH)):
                nc.sync.dma_start(out=wt_sb[:, kt, :], in_=wt_d.ap()[:, kt, :])

            # all embedding gathers issued upfront; they pipeline on the
            # dynamic DMA queue well ahead of the recurrence consuming them.
            xgs = []
            for g in range(NG):
                xg = gpool.tile([P, E], BF16, tag="xg")
                nc.gpsimd.indirect_dma_start(
                    out=xg[:],
                    out_offset=None,
                    in_=emb_d.ap(),
                    in_offset=IndirectOffsetOnAxis(ap=idx_sb[:, g:g + 1], axis=0),
                )
                xgs.append(xg)

            # PE-transpose one step's gather tiles into feature-major
            def emit_transposes(tt):
                for bb in range(NBT):
                    xg = xgs[tt * NBT + bb]
                    for e in range(KE):
                        ps_tr = pspool.tile([P, P], BF16, tag="ps",
                                            name="ps_tr")
                        nc.tensor.transpose(
                            ps_tr[:], xg[:, e * P:(e + 1) * P], ident[:])
                        nc.vector.tensor_copy(
                            out=xt_sb[:, tt, e, bb * P:(bb + 1) * P],
                            in_=ps_tr[:])

            # ---- LSTM recurrence ----
            emit_transposes(0)
            for t in range(T):
                rbuf, wbuf = t % 2, (t + 1) % 2
                # x-part k-tiles first: their rhs (xt) is ready immediately,
                # so PE enters the step while the tail of the previous
                # step's h writes is still in flight.
                ks = (list(range(KH, KHX)) + list(range(KH)) if t > 0
                      else list(range(KH, KHX)))

                for j in range(NJ):
                    # next step's transposes go mid-stream, where PSUM slots
                    # are freshly recycled — not at the step boundary where
                    # they'd contend with the previous step's gate drains
                    if j == 1 and t + 1 < T:
                        emit_transposes(t + 1)
                    gate_ps = []
                    for gi in range(4):
                        ps = pspool.tile([P, VN], F32, tag="ps")
                        col = gi * H + j * P
                        for n, k in enumerate(ks):
                            rhs = (h_sb[:, rbuf, k, :] if k < KH
                                   else xt_sb[:, t, k - KH, :])
                            nc.tensor.matmul(
                                ps[:],
                                lhsT=wt_sb[:, k, col:col + P],
                                rhs=rhs,
                                start=(n == 0),
                                stop=(n == len(ks) - 1),
                            )
                        gate_ps.append(ps)

                    bcol = lambda gi: bias_sb[:, gi * NJ + j:gi * NJ + j + 1]
                    f_sb = wpool.tile([P, BS], F32, tag="f")
                    i_sb = wpool.tile([P, BS], F32, tag="i")
                    g_sb = wpool.tile([P, BS], F32, tag="g")
                    o_sb = wpool.tile([P, BS], F32, tag="o")
                    nc.scalar.activation(f_sb[:], gate_ps[0][:], AF.Sigmoid,
                                         bias=bcol(0))
                    nc.scalar.activation(i_sb[:], gate_ps[1][:], AF.Sigmoid,
                                         bias=bcol(1))
                    nc.scalar.activation(g_sb[:], gate_ps[2][:], AF.Tanh,
                                         bias=bcol(2))
                    nc.scalar.activation(o_sb[:], gate_ps[3][:], AF.Sigmoid,
                                         bias=bcol(3))

                    if t == 0:
                        nc.vector.tensor_mul(out=c_sb[:, j, :], in0=i_sb[:],
                                             in1=g_sb[:])
                    else:
                        # in-place: c *= f; g_sb = i*g; c += g_sb
                        nc.vector.tensor_mul(out=c_sb[:, j, :], in0=f_sb[:],
                                             in1=c_sb[:, j, :])
                        nc.vector.tensor_mul(out=g_sb[:], in0=i_sb[:],
                                             in1=g_sb[:])
                        nc.vector.tensor_add(out=c_sb[:, j, :],
                                             in0=c_sb[:, j, :], in1=g_sb[:])
                    th = wpool.tile([P, BS], F32, tag="th")
                    nc.scalar.activation(th[:], c_sb[:, j, :], AF.Tanh)
                    nc.vector.tensor_mul(out=h_sb[:, wbuf, j, :], in0=o_sb[:],
                                         in1=th[:])

            # ---- output projection ----
            hbuf = T % 2
            QW = KH * VN // 4  # wout tile loaded in 4 quarters for overlap
            for vt in range(VT):
                vn = min(VN, V - vt * VN)
                wo_sb = wopool.tile([P, KH * VN], BF16, tag="wo")
                for q in range(4):
                    nc.sync.dma_start(out=wo_sb[:, q * QW:(q + 1) * QW],
                                      in_=wo_d.ap()[vt][:, q * QW:(q + 1) * QW])
                for bt in range(NBT):
                    ps = pspool.tile([P, VN], F32, tag="ps")
                    for k in range(KH):
                        nc.tensor.matmul(
                            ps[:, :vn],
                            lhsT=h_sb[:, hbuf, k, bt * P:(bt + 1) * P],
                            rhs=wo_sb[:, k * VN:k * VN + vn],
                            start=(k == 0),
                            stop=(k == KH - 1),
                        )
                    ot = opool.tile([P, VN], F32, tag="ot")
                    nc.vector.tensor_copy(out=ot[:, :vn], in_=ps[:, :vn])
                    # logit writes go out on the ACT HWDGE queue so they
                    # don't contend with the wout reads on the sync queue
                    nc.scalar.dma_start(
                        out=out_d.ap()[bt * P:(bt + 1) * P,
                                       vt * VN:vt * VN + vn],
                        in_=ot[:, :vn])

    nc.compile()
    return nc


def get_nc():
    if "nc" not in _CACHE:
        _CACHE["nc"] = _build()
    return _CACHE["nc"]


def _prep_shared(Emb, WF, WI, WC, WO, bF, bI, bC, bO, Wout):
    emb = np.ascontiguousarray(np.asarray(Emb, dtype=np.float32)).astype(_BF)

    WT = np.concatenate([np.asarray(WF), np.asarray(WI), np.asarray(WC),
                         np.asarray(WO)], 0).astype(np.float32).T  # [1536, 4096]
    wt = np.ascontiguousarray(
        WT.reshape(KHX, P, 4 * H).transpose(1, 0, 2)).astype(_BF)  # [128,12,4096]

    b_all = np.concatenate([np.asarray(bF), np.asarray(bI), np.asarray(bC),
                            np.asarray(bO)], 0).astype(np.float32)  # [4096]
    bias = np.ascontiguousarray(b_all.reshape(4 * H // P, P).T)  # [128, 32]

    Wout = np.asarray(Wout, dtype=np.float32)
    wpad = np.zeros((VPAD, H), np.float32)
    wpad[:V] = Wout
    wo = np.ascontiguousarray(
        wpad.reshape(VT, VN, KH, P).transpose(0, 3, 2, 1).reshape(VT, P, KH * VN)
    ).astype(_BF)  # [63, 128, 4096]
    return emb, wt, bias, wo


def kernel(X, Emb, WF, bF, WI, bI, WC, bC, WO, bO, Wout, bout):
    global LAST_RESULTS
    nc = get_nc()

    emb, wt, bias, wo = _prep_shared(Emb, WF, WI, WC, WO, bF, bI, bC, bO, Wout)
    X = np.asarray(X).astype(np.int32)  # [4096, 5]

    in_maps = []
    for c in range(NCORES):
        xs = X[c * BS:(c + 1) * BS]                       # [512, 5]
        idx = np.ascontiguousarray(
            xs.T.reshape(NG, P).T).astype(np.int32)       # [128, 20] t-major
        in_maps.append({"idx": idx, "emb": emb, "wt": wt,
                        "bias": bias, "wo": wo})

    res = run_bass_kernel_spmd(nc, in_maps, core_ids=list(range(NCORES)))
    LAST_RESULTS = res

    out = np.concatenate([res.results[c]["out"] for c in range(NCORES)], 0)
    bout = np.asarray(bout, dtype=np.float32)
    if np.any(bout):
        out = out + bout[None, :]
    return out



# revision 2
# speedup vs baseline: 1.3615x; 1.3615x over previous
"""TextLSTM kernel for 8 Trainium2 NeuronCores.

Data-parallel over batch: each of the 8 cores runs the full model on a
512-row batch shard.

Per-core pipeline (feature-major LSTM):
  0. PE warmup: ~48 identity matmuls run before ANY DMA is allowed to
     start (all initial loads are pinned behind the warmup via dummy
     writes).  If DMA is active while the PE ramps out of its low
     p-state, the PE clock latches at 2.0 GHz instead of 2.4 GHz for the
     whole execution (+20% on every matmul).  A second dummy-matmul block
     keeps the PE busy while the embedding gathers / weight loads land.
  1. Embedding gather: indirect-DMA 2560 rows of Emb (t-major token order)
     into SBUF batch-major, PE-transpose 128x128 blocks into feature-major
     xT[t] tiles (bf16 + fp8e4m3 copies, fp8 pre-scaled by 2^8).
  2. LSTM recurrence, 5 steps: f/i/o gate matmuls run in fp8 DoubleRow
     mode (2 k-tiles per instruction, 2x PE throughput; safe because the
     sigmoid pre-activations are tiny so quantization noise is damped
     ~100x), the g (tanh) gate and projection stay bf16.  Gates
     [4H, 512b] accumulated in fp32 PSUM; sigmoid/tanh on ScalarE with
     the 2^-16 fp8 descale + bias folded in; cell math fp32 on VectorE;
     h stored bf16 + fp8 (double-buffered), c fp32.
  3. Projection: out[512b, 32000v] = h.T @ WoutT streamed per 512-col
     vocab tile (bf16 weights, fp32 PSUM/output).

Weights are pre-transposed/tiled/cast on the host; biases are all zero per
the problem spec (gate biases are still applied on-device via the activation
bias port; bout is added on host only if nonzero).
"""

import os
import sys

import numpy as np
import ml_dtypes

for _p in ("/opt/trn_rl_repo", "/root/.axon_site/_ro/trn_rl_repo"):
    if os.path.isdir(_p) and _p not in sys.path:
        sys.path.append(_p)

from concourse import bacc, mybir
import concourse.tile as tile
from concourse.bass import IndirectOffsetOnAxis
from concourse.bass_utils import run_bass_kernel_spmd
from concourse.masks import make_identity

P = 128
B, T, E, H, V = 4096, 5, 512, 1024, 32000
NCORES = 8
BS = B // NCORES          # 512 batch rows per core
NTOK = BS * T             # 2560 gathered tokens per core
NG = NTOK // P            # 20 gather tiles of 128 tokens
KH = H // P               # 8 k-tiles over h
KE = E // P               # 4 k-tiles over x
KHX = KH + KE             # 12 k-tiles over [h; x]
NJ = H // P               # 8 hidden-dim tiles
VN = 512                  # vocab tile width
VT = (V + VN - 1) // VN   # 63 vocab tiles (last one 256 wide)
VPAD = VT * VN            # 32256
NBT = BS // P             # 4 batch tiles

F32 = mybir.dt.float32
BF16 = mybir.dt.bfloat16
FP8 = mybir.dt.float8e4
I32 = mybir.dt.int32
AF = mybir.ActivationFunctionType
DR = mybir.MatmulPerfMode.DoubleRow

_BF = ml_dtypes.bfloat16
_F8 = ml_dtypes.float8_e4m3

FP8_SCALE = 256.0         # 2^8 pre-scale on each fp8 operand
FP8_DESCALE = 1.0 / (FP8_SCALE * FP8_SCALE)

WARM1 = 48                # PE ramp matmuls before DMA release
WARM2 = 96                # PE filler matmuls while gathers/weights land

_CACHE = {}
LAST_RESULTS = None


def _build():
    nc = bacc.Bacc("TRN2", target_bir_lowering=False, debug=False,
                   num_devices=NCORES)

    idx_d = nc.dram_tensor("idx", [P, NG], I32, kind="ExternalInput")
    emb_d = nc.dram_tensor("emb", [V, E], BF16, kind="ExternalInput")
    wt8_d = nc.dram_tensor("wt8", [P, KHX, 3 * H], FP8, kind="ExternalInput")
    wtg_d = nc.dram_tensor("wtg", [P, KHX, H], BF16, kind="ExternalInput")
    bias_d = nc.dram_tensor("bias", [P, 4 * H // P], F32, kind="ExternalInput")
    wo_d = nc.dram_tensor("wo", [VT, P, KH * VN], BF16, kind="ExternalInput")
    out_d = nc.dram_tensor("out", [BS, V], F32, kind="ExternalOutput")

    with tile.TileContext(nc) as tc:
        with (
            tc.tile_pool(name="const", bufs=1) as cpool,
            tc.tile_pool(name="gather", bufs=6) as gpool,
            tc.tile_pool(name="work", bufs=2) as wpool,
            tc.tile_pool(name="woutp", bufs=3) as wopool,
            tc.tile_pool(name="outp", bufs=4) as opool,
            tc.tile_pool(name="psum", bufs=8, space="PSUM") as pspool,
        ):
            ident = cpool.tile([P, P], BF16, tag="ident")
            make_identity(nc, ident[:])
            scratch = cpool.tile([P, VN], BF16, tag="scratch")
            nc.vector.memset(scratch[:], 0)

            # persistent SBUF state
            wt8_sb = cpool.tile([P, KHX, 3 * H], FP8, tag="wt8")
            wtg_sb = cpool.tile([P, KHX, H], BF16, tag="wtg")
            bias_sb = cpool.tile([P, 4 * H // P], F32, tag="bias")
            h_sb = cpool.tile([P, 2, KH, BS], BF16, tag="h")
            h8_sb = cpool.tile([P, 2, KH, BS], FP8, tag="h8")
            c_sb = cpool.tile([P, NJ, BS], F32, tag="c")
            xt_sb = cpool.tile([P, T, KE, BS], BF16, tag="xt")
            x8_sb = cpool.tile([P, T, KE, BS], FP8, tag="x8")
            idx_sb = cpool.tile([P, NG], I32, tag="idx")

            # ---- PE warmup: ramp the clock with zero DMA in flight ----
            ps_w = pspool.tile([P, P], F32, tag="ps", name="ps_warm")
            for _ in range(WARM1):
                nc.tensor.matmul(ps_w[:], lhsT=ident[:], rhs=ident[:],
                                 start=True, stop=True)

            # pin every initial DMA behind the warmup: a dummy vector write
            # into each DMA destination creates a WAW dependency.
            def pin(dst_ap):
                nc.vector.tensor_copy(out=dst_ap, in_=ps_w[:, :1])

            pin(idx_sb.bitcast(F32)[:, :1])
            pin(bias_sb[:, :1])
            for kt in range(KHX):
                pin(wt8_sb[:, kt, :4].bitcast(F32)[:, :1])
                pin(wtg_sb[:, kt, :2].bitcast(F32)[:, :1])

            nc.sync.dma_start(out=idx_sb[:], in_=idx_d.ap())
            nc.sync.dma_start(out=bias_sb[:], in_=bias_d.ap())
            # x-part weights (k 8..11) first: they gate the t=0 matmuls,
            # the h-part loads overlap with t=0 compute.
            for kt in list(range(KH, KHX)) + list(range(KH)):
                nc.sync.dma_start(out=wt8_sb[:, kt, :], in_=wt8_d.ap()[:, kt, :])
                nc.sync.dma_start(out=wtg_sb[:, kt, :], in_=wtg_d.ap()[:, kt, :])

            # all embedding gathers issued upfront; they pipeline on the
            # dynamic DMA queue well ahead of the recurrence consuming them.
            xgs = []
            for g in range(NG):
                xg = gpool.tile([P, E], BF16, tag="xg")
                nc.gpsimd.indirect_dma_start(
                    out=xg[:],
                    out_offset=None,
                    in_=emb_d.ap(),
                    in_offset=IndirectOffsetOnAxis(ap=idx_sb[:, g:g + 1], axis=0),
                )
                xgs.append(xg)

            # keep the PE busy while the gathers/x-weights stream in, so it
            # never idles with DMA active (idle + active DMA re-derates).
            for _ in range(WARM2):
                nc.tensor.matmul(ps_w[:], lhsT=ident[:], rhs=scratch[:, :P],
                                 start=True, stop=True)

            # PE-transpose one step's gather tiles into feature-major
            def emit_transposes(tt):
                for bb in range(NBT):
                    xg = xgs[tt * NBT + bb]
                    for e in range(KE):
                        ps_tr = pspool.tile([P, P], BF16, tag="ps",
                                            name="ps_tr")
                        nc.tensor.transpose(
                            ps_tr[:], xg[:, e * P:(e + 1) * P], ident[:])
                        nc.vector.tensor_copy(
                            out=xt_sb[:, tt, e, bb * P:(bb + 1) * P],
                            in_=ps_tr[:])
                        nc.vector.tensor_scalar_mul(
                            x8_sb[:, tt, e, bb * P:(bb + 1) * P],
                            ps_tr[:], FP8_SCALE)

            # ---- LSTM recurrence ----
            emit_transposes(0)
            for t in range(T):
                rbuf, wbuf = t % 2, (t + 1) % 2
                # x-part k-pairs first: their rhs (x8/xt) is ready
                # immediately, so PE enters the step while the tail of the
                # previous step's h writes is still in flight.
                if t > 0:
                    prs = [(KH, 2), (KH + 2, 2), (0, 2), (2, 2), (4, 2),
                           (6, 2)]
                    ks_g = list(range(KH, KHX)) + list(range(KH))
                else:
                    prs = [(KH, 2), (KH + 2, 2)]
                    ks_g = list(range(KH, KHX))

                for j in range(NJ):
                    # next step's transposes go mid-stream, where PSUM slots
                    # are freshly recycled — not at the step boundary where
                    # they'd contend with the previous step's gate drains
                    if j == 1 and t + 1 < T:
                        emit_transposes(t + 1)

                    # f, i, o gates: fp8 DoubleRow (2 k-tiles / instr)
                    fio_ps = []
                    for gi in range(3):
                        ps = pspool.tile([P, VN], F32, tag="ps")
                        col = gi * H + j * P
                        for n, (k0, _) in enumerate(prs):
                            if k0 >= KH:
                                rhs = x8_sb[:, t, k0 - KH:k0 - KH + 2, :]
                            else:
                                rhs = h8_sb[:, rbuf, k0:k0 + 2, :]
                            nc.tensor.matmul(
                                ps[:],
                                lhsT=wt8_sb[:, k0:k0 + 2, col:col + P],
                                rhs=rhs,
                                start=(n == 0),
                                stop=(n == len(prs) - 1),
                                perf_mode=DR,
                            )
                        fio_ps.append(ps)

                    # g gate: bf16 (error passes straight through tanh)
                    g_ps = pspool.tile([P, VN], F32, tag="ps")
                    colg = j * P
                    for n, k in enumerate(ks_g):
                        rhs = (h_sb[:, rbuf, k, :] if k < KH
                               else xt_sb[:, t, k - KH, :])
                        nc.tensor.matmul(
                            g_ps[:],
                            lhsT=wtg_sb[:, k, colg:colg + P],
                            rhs=rhs,
                            start=(n == 0),
                            stop=(n == len(ks_g) - 1),
                        )

                    bcol = lambda gi: bias_sb[:, gi * NJ + j:gi * NJ + j + 1]
                    f_sb = wpool.tile([P, BS], F32, tag="f")
                    i_sb = wpool.tile([P, BS], F32, tag="i")
                    o_sb = wpool.tile([P, BS], F32, tag="o")
                    g_sb = wpool.tile([P, BS], F32, tag="g")
                    nc.scalar.activation(f_sb[:], fio_ps[0][:], AF.Sigmoid,
                                         bias=bcol(0), scale=FP8_DESCALE)
                    nc.scalar.activation(i_sb[:], fio_ps[1][:], AF.Sigmoid,
                                         bias=bcol(1), scale=FP8_DESCALE)
                    nc.scalar.activation(o_sb[:], fio_ps[2][:], AF.Sigmoid,
                                         bias=bcol(3), scale=FP8_DESCALE)
                    nc.scalar.activation(g_sb[:], g_ps[:], AF.Tanh,
                                         bias=bcol(2))

                    if t == 0:
                        nc.vector.tensor_mul(out=c_sb[:, j, :], in0=i_sb[:],
                                             in1=g_sb[:])
                    else:
                        # in-place: c *= f; g_sb = i*g; c += g_sb
                        nc.vector.tensor_mul(out=c_sb[:, j, :], in0=f_sb[:],
                                             in1=c_sb[:, j, :])
                        nc.vector.tensor_mul(out=g_sb[:], in0=i_sb[:],
                                             in1=g_sb[:])
                        nc.vector.tensor_add(out=c_sb[:, j, :],
                                             in0=c_sb[:, j, :], in1=g_sb[:])
                    th = wpool.tile([P, BS], F32, tag="th")
                    nc.scalar.activation(th[:], c_sb[:, j, :], AF.Tanh)
                    nc.vector.tensor_mul(out=h_sb[:, wbuf, j, :], in0=o_sb[:],
                                         in1=th[:])
                    if t + 1 < T:
                        nc.vector.tensor_scalar_mul(
                            h8_sb[:, wbuf, j, :], h_sb[:, wbuf, j, :],
                            FP8_SCALE)

            # ---- output projection ----
            hbuf = T % 2
            QW = KH * VN // 4  # wout tile loaded in 4 quarters for overlap
            for vt in range(VT):
                vn = min(VN, V - vt * VN)
                wo_sb = wopool.tile([P, KH * VN], BF16, tag="wo")
                for q in range(4):
                    nc.sync.dma_start(out=wo_sb[:, q * QW:(q + 1) * QW],
                                      in_=wo_d.ap()[vt][:, q * QW:(q + 1) * QW])
                for bt in range(NBT):
                    ps = pspool.tile([P, VN], F32, tag="ps")
                    for k in range(KH):
                        nc.tensor.matmul(
                            ps[:, :vn],
                            lhsT=h_sb[:, hbuf, k, bt * P:(bt + 1) * P],
                            rhs=wo_sb[:, k * VN:k * VN + vn],
                            start=(k == 0),
                            stop=(k == KH - 1),
                        )
                    ot = opool.tile([P, VN], F32, tag="ot")
                    nc.vector.tensor_copy(out=ot[:, :vn], in_=ps[:, :vn])
                    # logit writes go out on the ACT HWDGE queue so they
                    # don't contend with the wout reads on the sync queue
                    nc.scalar.dma_start(
                        out=out_d.ap()[bt * P:(bt + 1) * P,
                                       vt * VN:vt * VN + vn],
                        in_=ot[:, :vn])

    nc.compile()
    return nc


def get_nc():
    if "nc" not in _CACHE:
        _CACHE["nc"] = _build()
    return _CACHE["nc"]


def _prep_shared(Emb, WF, WI, WC, WO, bF, bI, bC, bO, Wout):
    emb = np.ascontiguousarray(np.asarray(Emb, dtype=np.float32)).astype(_BF)

    # f/i/o gate weights: fp8 e4m3, pre-scaled by 2^8
    W3 = np.concatenate([np.asarray(WF), np.asarray(WI), np.asarray(WO)],
                        0).astype(np.float32).T          # [1536, 3072]
    wt8 = np.ascontiguousarray(
        W3.reshape(KHX, P, 3 * H).transpose(1, 0, 2))    # [128, 12, 3072]
    wt8 = np.clip(wt8 * FP8_SCALE, -240.0, 240.0).astype(_F8)

    # g gate weights: bf16
    WG = np.asarray(WC).astype(np.float32).T             # [1536, 1024]
    wtg = np.ascontiguousarray(
        WG.reshape(KHX, P, H).transpose(1, 0, 2)).astype(_BF)  # [128, 12, 1024]

    b_all = np.concatenate([np.asarray(bF), np.asarray(bI), np.asarray(bC),
                            np.asarray(bO)], 0).astype(np.float32)  # [4096]
    bias = np.ascontiguousarray(b_all.reshape(4 * H // P, P).T)  # [128, 32]

    Wout = np.asarray(Wout, dtype=np.float32)
    wpad = np.zeros((VPAD, H), np.float32)
    wpad[:V] = Wout
    wo = np.ascontiguousarray(
        wpad.reshape(VT, VN, KH, P).transpose(0, 3, 2, 1).reshape(VT, P, KH * VN)
    ).astype(_BF)  # [63, 128, 4096]
    return emb, wt8, wtg, bias, wo


def kernel(X, Emb, WF, bF, WI, bI, WC, bC, WO, bO, Wout, bout):
    global LAST_RESULTS
    nc = get_nc()

    emb, wt8, wtg, bias, wo = _prep_shared(Emb, WF, WI, WC, WO, bF, bI, bC,
                                           bO, Wout)
    X = np.asarray(X).astype(np.int32)  # [4096, 5]

    in_maps = []
    for c in range(NCORES):
        xs = X[c * BS:(c + 1) * BS]                       # [512, 5]
        idx = np.ascontiguousarray(
            xs.T.reshape(NG, P).T).astype(np.int32)       # [128, 20] t-major
        in_maps.append({"idx": idx, "emb": emb, "wt8": wt8, "wtg": wtg,
                        "bias": bias, "wo": wo})

    res = run_bass_kernel_spmd(nc, in_maps, core_ids=list(range(NCORES)))
    LAST_RESULTS = res

    out = np.concatenate([res.results[c]["out"] for c in range(NCORES)], 0)
    bout = np.asarray(bout, dtype=np.float32)
    if np.any(bout):
        out = out + bout[None, :]
    return out


# revision 4
# speedup vs baseline: 1.3756x; 1.0104x over previous
"""TextLSTM kernel for 8 Trainium2 NeuronCores.

Data-parallel over batch: each of the 8 cores runs the full model on a
512-row batch shard.

Per-core pipeline (feature-major LSTM):
  0. PE warmup: ~48 identity matmuls run before ANY DMA is allowed to
     start (all initial loads are pinned behind the warmup via dummy
     writes).  If DMA is active while the PE ramps out of its low
     p-state, the PE clock latches at 2.0 GHz instead of 2.4 GHz for the
     whole execution (+20% on every matmul).  A second dummy-matmul block
     keeps the PE busy while the embedding gathers / weight loads land.
  1. Embedding gather: indirect-DMA 2560 rows of Emb (t-major token order)
     into SBUF batch-major, PE-transpose 128x128 blocks into feature-major
     xT[t] tiles (bf16 + fp8e4m3 copies, fp8 pre-scaled by 2^8).
  2. LSTM recurrence, 5 steps: f/i/o gate matmuls run in fp8 DoubleRow
     mode (2 k-tiles per instruction, 2x PE throughput; safe because the
     sigmoid pre-activations are tiny so quantization noise is damped
     ~100x), the g (tanh) gate and projection stay bf16.  Gates
     [4H, 512b] accumulated in fp32 PSUM; sigmoid/tanh on ScalarE with
     the 2^-16 fp8 descale + bias folded in; cell math fp32 on VectorE;
     h stored bf16 + fp8 (double-buffered), c fp32.
  3. Projection: out[512b, 32000v] = h.T @ WoutT streamed per 512-col
     vocab tile (bf16 weights, fp32 PSUM/output).

Weights are pre-transposed/tiled/cast on the host; biases are all zero per
the problem spec (gate biases are still applied on-device via the activation
bias port; bout is added on host only if nonzero).
"""

import os
import sys

import numpy as np
import ml_dtypes

for _p in ("/opt/trn_rl_repo", "/root/.axon_site/_ro/trn_rl_repo"):
    if os.path.isdir(_p) and _p not in sys.path:
        sys.path.append(_p)

from concourse import bacc, mybir
import concourse.tile as tile
from concourse.bass import IndirectOffsetOnAxis
from concourse.bass_utils import run_bass_kernel_spmd
from concourse.masks import make_identity

P = 128
B, T, E, H, V = 4096, 5, 512, 1024, 32000
NCORES = 8
BS = B // NCORES          # 512 batch rows per core
NTOK = BS * T             # 2560 gathered tokens per core
NG = NTOK // P            # 20 gather tiles of 128 tokens
KH = H // P               # 8 k-tiles over h
KE = E // P               # 4 k-tiles over x
KHX = KH + KE             # 12 k-tiles over [h; x]
NJ = H // P               # 8 hidden-dim tiles
VN = 512                  # vocab tile width
VT = (V + VN - 1) // VN   # 63 vocab tiles (last one 256 wide)
VPAD = VT * VN            # 32256
NBT = BS // P             # 4 batch tiles

F32 = mybir.dt.float32
BF16 = mybir.dt.bfloat16
FP8 = mybir.dt.float8e4
I32 = mybir.dt.int32
AF = mybir.ActivationFunctionType
DR = mybir.MatmulPerfMode.DoubleRow

_BF = ml_dtypes.bfloat16
_F8 = ml_dtypes.float8_e4m3

FP8_SCALE = 256.0         # 2^8 pre-scale on each fp8 operand
FP8_DESCALE = 1.0 / (FP8_SCALE * FP8_SCALE)

WARM1 = 24                # PE ramp matmuls before big-DMA release
WARM2 = 64                # PE filler matmuls while gathers/weights land

_CACHE = {}
LAST_RESULTS = None


def _build():
    nc = bacc.Bacc("TRN2", target_bir_lowering=False, debug=False,
                   num_devices=NCORES)

    idx_d = nc.dram_tensor("idx", [P, NG], I32, kind="ExternalInput")
    emb_d = nc.dram_tensor("emb", [V, E], BF16, kind="ExternalInput")
    wt8_d = nc.dram_tensor("wt8", [P, KHX, 3 * H], FP8, kind="ExternalInput")
    wtg_d = nc.dram_tensor("wtg", [P, KHX, H], BF16, kind="ExternalInput")
    bias_d = nc.dram_tensor("bias", [P, 4 * H // P], F32, kind="ExternalInput")
    wo_d = nc.dram_tensor("wo", [VT, P, KH * VN], BF16, kind="ExternalInput")
    out_d = nc.dram_tensor("out", [BS, V], F32, kind="ExternalOutput")

    with tile.TileContext(nc) as tc:
        with (
            tc.tile_pool(name="const", bufs=1) as cpool,
            tc.tile_pool(name="gather", bufs=6) as gpool,
            tc.tile_pool(name="work", bufs=2) as wpool,
            tc.tile_pool(name="woutp", bufs=3) as wopool,
            tc.tile_pool(name="outp", bufs=4) as opool,
            tc.tile_pool(name="psum", bufs=8, space="PSUM") as pspool,
        ):
            ident = cpool.tile([P, P], BF16, tag="ident")
            make_identity(nc, ident[:])
            scratch = cpool.tile([P, VN], BF16, tag="scratch")
            nc.vector.memset(scratch[:], 0)

            # persistent SBUF state
            wt8_sb = cpool.tile([P, KHX, 3 * H], FP8, tag="wt8")
            wtg_sb = cpool.tile([P, KHX, H], BF16, tag="wtg")
            bias_sb = cpool.tile([P, 4 * H // P], F32, tag="bias")
            h_sb = cpool.tile([P, 2, KH, BS], BF16, tag="h")
            h8_sb = cpool.tile([P, 2, KH, BS], FP8, tag="h8")
            c_sb = cpool.tile([P, NJ, BS], F32, tag="c")
            xt_sb = cpool.tile([P, T, KE, BS], BF16, tag="xt")
            x8_sb = cpool.tile([P, T, KE, BS], FP8, tag="x8")
            idx_sb = cpool.tile([P, NG], I32, tag="idx")

            # small-descriptor DMAs (idx load + embedding gathers) are
            # latch-safe and start immediately; only the big weight-chunk
            # loads must wait for the PE clock ramp.
            nc.sync.dma_start(out=idx_sb[:], in_=idx_d.ap())
            nc.sync.dma_start(out=bias_sb[:], in_=bias_d.ap())
            xgs = []
            for g in range(NG):
                xg = gpool.tile([P, E], BF16, tag="xg")
                nc.gpsimd.indirect_dma_start(
                    out=xg[:],
                    out_offset=None,
                    in_=emb_d.ap(),
                    in_offset=IndirectOffsetOnAxis(ap=idx_sb[:, g:g + 1], axis=0),
                )
                xgs.append(xg)

            # ---- PE warmup: ramp the clock before the big DMAs start ----
            ps_w = pspool.tile([P, P], F32, tag="ps", name="ps_warm")
            for _ in range(WARM1):
                nc.tensor.matmul(ps_w[:], lhsT=ident[:], rhs=ident[:],
                                 start=True, stop=True)

            # pin the weight loads behind the warmup: a dummy vector write
            # into each DMA destination creates a WAW dependency.
            def pin(dst_ap):
                nc.vector.tensor_copy(out=dst_ap, in_=ps_w[:, :1])

            # x-part weights (k 8..11) first: they gate the t=0 matmuls,
            # the h-part loads overlap with t=0 compute.
            for kt in list(range(KH, KHX)) + list(range(KH)):
                pin(wt8_sb[:, kt, :4].bitcast(F32)[:, :1])
                pin(wtg_sb[:, kt, :2].bitcast(F32)[:, :1])
                nc.sync.dma_start(out=wt8_sb[:, kt, :], in_=wt8_d.ap()[:, kt, :])
                nc.sync.dma_start(out=wtg_sb[:, kt, :], in_=wtg_d.ap()[:, kt, :])

            # keep the PE busy while the gathers/x-weights stream in, so it
            # never idles with DMA active (idle + active DMA re-derates).
            ps_w2 = pspool.tile([P, P], F32, tag="ps", name="ps_warm2")
            for _ in range(WARM2):
                nc.tensor.matmul(ps_w2[:], lhsT=ident[:], rhs=scratch[:, :P],
                                 start=True, stop=True)

            # PE-transpose one step's gather tiles into feature-major
            def emit_transposes(tt):
                for bb in range(NBT):
                    xg = xgs[tt * NBT + bb]
                    for e in range(KE):
                        ps_tr = pspool.tile([P, P], BF16, tag="ps",
                                            name="ps_tr")
                        nc.tensor.transpose(
                            ps_tr[:], xg[:, e * P:(e + 1) * P], ident[:])
                        nc.vector.tensor_copy(
                            out=xt_sb[:, tt, e, bb * P:(bb + 1) * P],
                            in_=ps_tr[:])
                        nc.vector.tensor_scalar_mul(
                            x8_sb[:, tt, e, bb * P:(bb + 1) * P],
                            ps_tr[:], FP8_SCALE)

            # ---- LSTM recurrence ----
            emit_transposes(0)
            for t in range(T):
                rbuf, wbuf = t % 2, (t + 1) % 2
                # x-part k-pairs first: their rhs (x8/xt) is ready
                # immediately, so PE enters the step while the tail of the
                # previous step's h writes is still in flight.
                if t > 0:
                    prs = [(KH, 2), (KH + 2, 2), (0, 2), (2, 2), (4, 2),
                           (6, 2)]
                    ks_g = list(range(KH, KHX)) + list(range(KH))
                else:
                    prs = [(KH, 2), (KH + 2, 2)]
                    ks_g = list(range(KH, KHX))

                for j in range(NJ):
                    # next step's transposes go mid-stream, where PSUM slots
                    # are freshly recycled — not at the step boundary where
                    # they'd contend with the previous step's gate drains
                    if j == 1 and t + 1 < T:
                        emit_transposes(t + 1)

                    # f, i, o gates: fp8 DoubleRow (2 k-tiles / instr)
                    fio_ps = []
                    for gi in range(3):
                        ps = pspool.tile([P, VN], F32, tag="ps")
                        col = gi * H + j * P
                        for n, (k0, _) in enumerate(prs):
                            if k0 >= KH:
                                rhs = x8_sb[:, t, k0 - KH:k0 - KH + 2, :]
                            else:
                                rhs = h8_sb[:, rbuf, k0:k0 + 2, :]
                            nc.tensor.matmul(
                                ps[:],
                                lhsT=wt8_sb[:, k0:k0 + 2, col:col + P],
                                rhs=rhs,
                                start=(n == 0),
                                stop=(n == len(prs) - 1),
                                perf_mode=DR,
                            )
                        fio_ps.append(ps)

                    # g gate: bf16 (error passes straight through tanh)
                    g_ps = pspool.tile([P, VN], F32, tag="ps")
                    colg = j * P
                    for n, k in enumerate(ks_g):
                        rhs = (h_sb[:, rbuf, k, :] if k < KH
                               else xt_sb[:, t, k - KH, :])
                        nc.tensor.matmul(
                            g_ps[:],
                            lhsT=wtg_sb[:, k, colg:colg + P],
                            rhs=rhs,
                            start=(n == 0),
                            stop=(n == len(ks_g) - 1),
                        )

                    bcol = lambda gi: bias_sb[:, gi * NJ + j:gi * NJ + j + 1]
                    f_sb = wpool.tile([P, BS], F32, tag="f")
                    i_sb = wpool.tile([P, BS], F32, tag="i")
                    o_sb = wpool.tile([P, BS], F32, tag="o")
                    g_sb = wpool.tile([P, BS], F32, tag="g")
                    nc.scalar.activation(f_sb[:], fio_ps[0][:], AF.Sigmoid,
                                         bias=bcol(0), scale=FP8_DESCALE)
                    nc.scalar.activation(i_sb[:], fio_ps[1][:], AF.Sigmoid,
                                         bias=bcol(1), scale=FP8_DESCALE)
                    nc.scalar.activation(o_sb[:], fio_ps[2][:], AF.Sigmoid,
                                         bias=bcol(3), scale=FP8_DESCALE)
                    nc.scalar.activation(g_sb[:], g_ps[:], AF.Tanh,
                                         bias=bcol(2))

                    if t == 0:
                        nc.vector.tensor_mul(out=c_sb[:, j, :], in0=i_sb[:],
                                             in1=g_sb[:])
                    else:
                        # in-place: c *= f; g_sb = i*g; c += g_sb
                        nc.vector.tensor_mul(out=c_sb[:, j, :], in0=f_sb[:],
                                             in1=c_sb[:, j, :])
                        nc.vector.tensor_mul(out=g_sb[:], in0=i_sb[:],
                                             in1=g_sb[:])
                        nc.vector.tensor_add(out=c_sb[:, j, :],
                                             in0=c_sb[:, j, :], in1=g_sb[:])
                    th = wpool.tile([P, BS], F32, tag="th")
                    nc.scalar.activation(th[:], c_sb[:, j, :], AF.Tanh)
                    nc.vector.tensor_mul(out=h_sb[:, wbuf, j, :], in0=o_sb[:],
                                         in1=th[:])
                    if t + 1 < T:
                        nc.vector.tensor_scalar_mul(
                            h8_sb[:, wbuf, j, :], h_sb[:, wbuf, j, :],
                            FP8_SCALE)

            # ---- output projection ----
            hbuf = T % 2
            QW = KH * VN // 4  # wout tile loaded in 4 quarters for overlap
            for vt in range(VT):
                vn = min(VN, V - vt * VN)
                wo_sb = wopool.tile([P, KH * VN], BF16, tag="wo")
                for q in range(4):
                    nc.sync.dma_start(out=wo_sb[:, q * QW:(q + 1) * QW],
                                      in_=wo_d.ap()[vt][:, q * QW:(q + 1) * QW])
                for bt in range(NBT):
                    ps = pspool.tile([P, VN], F32, tag="ps")
                    for k in range(KH):
                        nc.tensor.matmul(
                            ps[:, :vn],
                            lhsT=h_sb[:, hbuf, k, bt * P:(bt + 1) * P],
                            rhs=wo_sb[:, k * VN:k * VN + vn],
                            start=(k == 0),
                            stop=(k == KH - 1),
                        )
                    ot = opool.tile([P, VN], F32, tag="ot")
                    nc.vector.tensor_copy(out=ot[:, :vn], in_=ps[:, :vn])
                    # logit writes go out on the ACT HWDGE queue so they
                    # don't contend with the wout reads on the sync queue
                    nc.scalar.dma_start(
                        out=out_d.ap()[bt * P:(bt + 1) * P,
                                       vt * VN:vt * VN + vn],
                        in_=ot[:, :vn])

    nc.compile()
    return nc


def get_nc():
    if "nc" not in _CACHE:
        _CACHE["nc"] = _build()
    return _CACHE["nc"]


def _prep_shared(Emb, WF, WI, WC, WO, bF, bI, bC, bO, Wout):
    emb = np.ascontiguousarray(np.asarray(Emb, dtype=np.float32)).astype(_BF)

    # f/i/o gate weights: fp8 e4m3, pre-scaled by 2^8
    W3 = np.concatenate([np.asarray(WF), np.asarray(WI), np.asarray(WO)],
                        0).astype(np.float32).T          # [1536, 3072]
    wt8 = np.ascontiguousarray(
        W3.reshape(KHX, P, 3 * H).transpose(1, 0, 2))    # [128, 12, 3072]
    wt8 = np.clip(wt8 * FP8_SCALE, -240.0, 240.0).astype(_F8)

    # g gate weights: bf16
    WG = np.asarray(WC).astype(np.float32).T             # [1536, 1024]
    wtg = np.ascontiguousarray(
        WG.reshape(KHX, P, H).transpose(1, 0, 2)).astype(_BF)  # [128, 12, 1024]

    b_all = np.concatenate([np.asarray(bF), np.asarray(bI), np.asarray(bC),
                            np.asarray(bO)], 0).astype(np.float32)  # [4096]
    bias = np.ascontiguousarray(b_all.reshape(4 * H // P, P).T)  # [128, 32]

    Wout = np.asarray(Wout, dtype=np.float32)
    wpad = np.zeros((VPAD, H), np.float32)
    wpad[:V] = Wout
    wo = np.ascontiguousarray(
        wpad.reshape(VT, VN, KH, P).transpose(0, 3, 2, 1).reshape(VT, P, KH * VN)
    ).astype(_BF)  # [63, 128, 4096]
    return emb, wt8, wtg, bias, wo


def kernel(X, Emb, WF, bF, WI, bI, WC, bC, WO, bO, Wout, bout):
    global LAST_RESULTS
    nc = get_nc()

    emb, wt8, wtg, bias, wo = _prep_shared(Emb, WF, WI, WC, WO, bF, bI, bC,
                                           bO, Wout)
    X = np.asarray(X).astype(np.int32)  # [4096, 5]

    in_maps = []
    for c in range(NCORES):
        xs = X[c * BS:(c + 1) * BS]                       # [512, 5]
        idx = np.ascontiguousarray(
            xs.T.reshape(NG, P).T).astype(np.int32)       # [128, 20] t-major
        in_maps.append({"idx": idx, "emb": emb, "wt8": wt8, "wtg": wtg,
                        "bias": bias, "wo": wo})

    res = run_bass_kernel_spmd(nc, in_maps, core_ids=list(range(NCORES)))
    LAST_RESULTS = res

    out = np.concatenate([res.results[c]["out"] for c in range(NCORES)], 0)
    bout = np.asarray(bout, dtype=np.float32)
    if np.any(bout):
        out = out + bout[None, :]
    return out


# revision 6
# speedup vs baseline: 1.3758x; 1.0001x over previous
"""TextLSTM kernel for 8 Trainium2 NeuronCores.

Data-parallel over batch: each of the 8 cores runs the full model on a
512-row batch shard.

Per-core pipeline (feature-major LSTM):
  0. PE warmup: ~48 identity matmuls run before ANY DMA is allowed to
     start (all initial loads are pinned behind the warmup via dummy
     writes).  If DMA is active while the PE ramps out of its low
     p-state, the PE clock latches at 2.0 GHz instead of 2.4 GHz for the
     whole execution (+20% on every matmul).  A second dummy-matmul block
     keeps the PE busy while the embedding gathers / weight loads land.
  1. Embedding gather: indirect-DMA 2560 rows of Emb (t-major token order)
     into SBUF batch-major, PE-transpose 128x128 blocks into feature-major
     xT[t] tiles (bf16 + fp8e4m3 copies, fp8 pre-scaled by 2^8).
  2. LSTM recurrence, 5 steps: f/i/o gate matmuls run in fp8 DoubleRow
     mode (2 k-tiles per instruction, 2x PE throughput; safe because the
     sigmoid pre-activations are tiny so quantization noise is damped
     ~100x), the g (tanh) gate and projection stay bf16.  Gates
     [4H, 512b] accumulated in fp32 PSUM; sigmoid/tanh on ScalarE with
     the 2^-16 fp8 descale + bias folded in; cell math fp32 on VectorE;
     h stored bf16 + fp8 (double-buffered), c fp32.
  3. Projection: out[512b, 32000v] = h.T @ WoutT streamed per 512-col
     vocab tile (bf16 weights, fp32 PSUM/output).

Weights are pre-transposed/tiled/cast on the host; biases are all zero per
the problem spec (gate biases are still applied on-device via the activation
bias port; bout is added on host only if nonzero).
"""

import os
import sys

import numpy as np
import ml_dtypes

for _p in ("/opt/trn_rl_repo", "/root/.axon_site/_ro/trn_rl_repo"):
    if os.path.isdir(_p) and _p not in sys.path:
        sys.path.append(_p)

from concourse import bacc, mybir
import concourse.tile as tile
from concourse.bass import IndirectOffsetOnAxis
from concourse.bass_utils import run_bass_kernel_spmd
from concourse.masks import make_identity

P = 128
B, T, E, H, V = 4096, 5, 512, 1024, 32000
NCORES = 8
BS = B // NCORES          # 512 batch rows per core
NTOK = BS * T             # 2560 gathered tokens per core
NG = NTOK // P            # 20 gather tiles of 128 tokens
KH = H // P               # 8 k-tiles over h
KE = E // P               # 4 k-tiles over x
KHX = KH + KE             # 12 k-tiles over [h; x]
NJ = H // P               # 8 hidden-dim tiles
VN = 512                  # vocab tile width
VT = (V + VN - 1) // VN   # 63 vocab tiles (last one 256 wide)
VPAD = VT * VN            # 32256
NBT = BS // P             # 4 batch tiles

F32 = mybir.dt.float32
BF16 = mybir.dt.bfloat16
FP8 = mybir.dt.float8e4
I32 = mybir.dt.int32
AF = mybir.ActivationFunctionType
DR = mybir.MatmulPerfMode.DoubleRow

_BF = ml_dtypes.bfloat16
_F8 = ml_dtypes.float8_e4m3

FP8_SCALE = 256.0         # 2^8 pre-scale on each fp8 operand
FP8_DESCALE = 1.0 / (FP8_SCALE * FP8_SCALE)

WARM1 = 24                # PE ramp matmuls before big-DMA release
WARM2 = 64                # PE filler matmuls while gathers/weights land

_CACHE = {}
LAST_RESULTS = None


def _build():
    nc = bacc.Bacc("TRN2", target_bir_lowering=False, debug=False,
                   num_devices=NCORES)

    idx_d = nc.dram_tensor("idx", [P, NG], I32, kind="ExternalInput")
    emb_d = nc.dram_tensor("emb", [V, E], BF16, kind="ExternalInput")
    wt8_d = nc.dram_tensor("wt8", [P, KHX, 3 * H], FP8, kind="ExternalInput")
    wtg_d = nc.dram_tensor("wtg", [P, KHX, H], BF16, kind="ExternalInput")
    bias_d = nc.dram_tensor("bias", [P, 4 * H // P], F32, kind="ExternalInput")
    wo_d = nc.dram_tensor("wo", [VT, P, KH * VN], BF16, kind="ExternalInput")
    out_d = nc.dram_tensor("out", [BS, V], F32, kind="ExternalOutput")

    with tile.TileContext(nc) as tc:
        with (
            tc.tile_pool(name="const", bufs=1) as cpool,
            tc.tile_pool(name="gather", bufs=6) as gpool,
            tc.tile_pool(name="work", bufs=2) as wpool,
            tc.tile_pool(name="woutp", bufs=3) as wopool,
            tc.tile_pool(name="outp", bufs=4) as opool,
            tc.tile_pool(name="psum", bufs=8, space="PSUM") as pspool,
        ):
            ident = cpool.tile([P, P], BF16, tag="ident")
            make_identity(nc, ident[:])
            scratch = cpool.tile([P, VN], BF16, tag="scratch")
            nc.vector.memset(scratch[:], 0)

            # persistent SBUF state
            wt8_sb = cpool.tile([P, KHX, 3 * H], FP8, tag="wt8")
            wtg_sb = cpool.tile([P, KHX, H], BF16, tag="wtg")
            bias_sb = cpool.tile([P, 4 * H // P], F32, tag="bias")
            h_sb = cpool.tile([P, 2, KH, BS], BF16, tag="h")
            h8_sb = cpool.tile([P, 2, KH, BS], FP8, tag="h8")
            c_sb = cpool.tile([P, NJ, BS], F32, tag="c")
            xt_sb = cpool.tile([P, T, KE, BS], BF16, tag="xt")
            x8_sb = cpool.tile([P, T, KE, BS], FP8, tag="x8")
            idx_sb = cpool.tile([P, NG], I32, tag="idx")

            # small-descriptor DMAs (idx load + embedding gathers) are
            # latch-safe and start immediately; only the big weight-chunk
            # loads must wait for the PE clock ramp.
            nc.sync.dma_start(out=idx_sb[:], in_=idx_d.ap())
            nc.sync.dma_start(out=bias_sb[:], in_=bias_d.ap())
            xgs = []
            for g in range(NG):
                xg = gpool.tile([P, E], BF16, tag="xg")
                nc.gpsimd.indirect_dma_start(
                    out=xg[:],
                    out_offset=None,
                    in_=emb_d.ap(),
                    in_offset=IndirectOffsetOnAxis(ap=idx_sb[:, g:g + 1], axis=0),
                )
                xgs.append(xg)

            # ---- PE warmup: ramp the clock before the big DMAs start ----
            ps_w = pspool.tile([P, P], F32, tag="ps", name="ps_warm")
            for _ in range(WARM1):
                nc.tensor.matmul(ps_w[:], lhsT=ident[:], rhs=ident[:],
                                 start=True, stop=True)

            # pin the weight loads behind the warmup: a dummy vector write
            # into each DMA destination creates a WAW dependency.
            def pin(dst_ap):
                nc.vector.tensor_copy(out=dst_ap, in_=ps_w[:, :1])

            # x-part weights (k 8..11) first: they gate the t=0 matmuls,
            # the h-part loads overlap with t=0 compute.
            for kt in list(range(KH, KHX)) + list(range(KH)):
                pin(wt8_sb[:, kt, :4].bitcast(F32)[:, :1])
                pin(wtg_sb[:, kt, :2].bitcast(F32)[:, :1])
                nc.sync.dma_start(out=wt8_sb[:, kt, :], in_=wt8_d.ap()[:, kt, :])
                nc.sync.dma_start(out=wtg_sb[:, kt, :], in_=wtg_d.ap()[:, kt, :])

            # keep the PE busy while the gathers/x-weights stream in, so it
            # never idles with DMA active (idle + active DMA re-derates).
            ps_w2 = pspool.tile([P, P], F32, tag="ps", name="ps_warm2")
            for _ in range(WARM2):
                nc.tensor.matmul(ps_w2[:], lhsT=ident[:], rhs=scratch[:, :P],
                                 start=True, stop=True)

            # PE-transpose one step's gather tiles into feature-major
            def emit_transposes(tt):
                for bb in range(NBT):
                    xg = xgs[tt * NBT + bb]
                    for e in range(KE):
                        ps_tr = pspool.tile([P, P], BF16, tag="ps",
                                            name="ps_tr")
                        nc.tensor.transpose(
                            ps_tr[:], xg[:, e * P:(e + 1) * P], ident[:])
                        nc.vector.tensor_copy(
                            out=xt_sb[:, tt, e, bb * P:(bb + 1) * P],
                            in_=ps_tr[:])
                        nc.vector.tensor_scalar_mul(
                            x8_sb[:, tt, e, bb * P:(bb + 1) * P],
                            ps_tr[:], FP8_SCALE)

            # ---- LSTM recurrence ----
            emit_transposes(0)
            for t in range(T):
                rbuf, wbuf = t % 2, (t + 1) % 2
                # x-part k-pairs first: their rhs (x8/xt) is ready
                # immediately, so PE enters the step while the tail of the
                # previous step's h writes is still in flight.
                if t > 0:
                    prs = [(KH, 2), (KH + 2, 2), (0, 2), (2, 2), (4, 2),
                           (6, 2)]
                    ks_g = list(range(KH, KHX)) + list(range(KH))
                else:
                    prs = [(KH, 2), (KH + 2, 2)]
                    ks_g = list(range(KH, KHX))

                for j in range(NJ):
                    # next step's transposes go mid-stream, where PSUM slots
                    # are freshly recycled — not at the step boundary where
                    # they'd contend with the previous step's gate drains
                    if j == 1 and t + 1 < T:
                        emit_transposes(t + 1)

                    # f, i, o gates: fp8 DoubleRow (2 k-tiles / instr)
                    fio_ps = []
                    for gi in range(3):
                        ps = pspool.tile([P, VN], F32, tag="ps")
                        col = gi * H + j * P
                        for n, (k0, _) in enumerate(prs):
                            if k0 >= KH:
                                rhs = x8_sb[:, t, k0 - KH:k0 - KH + 2, :]
                            else:
                                rhs = h8_sb[:, rbuf, k0:k0 + 2, :]
                            nc.tensor.matmul(
                                ps[:],
                                lhsT=wt8_sb[:, k0:k0 + 2, col:col + P],
                                rhs=rhs,
                                start=(n == 0),
                                stop=(n == len(prs) - 1),
                                perf_mode=DR,
                            )
                        fio_ps.append(ps)

                    # g gate: bf16 (error passes straight through tanh)
                    g_ps = pspool.tile([P, VN], F32, tag="ps")
                    colg = j * P
                    for n, k in enumerate(ks_g):
                        rhs = (h_sb[:, rbuf, k, :] if k < KH
                               else xt_sb[:, t, k - KH, :])
                        nc.tensor.matmul(
                            g_ps[:],
                            lhsT=wtg_sb[:, k, colg:colg + P],
                            rhs=rhs,
                            start=(n == 0),
                            stop=(n == len(ks_g) - 1),
                        )

                    bcol = lambda gi: bias_sb[:, gi * NJ + j:gi * NJ + j + 1]
                    f_sb = wpool.tile([P, BS], F32, tag="f")
                    i_sb = wpool.tile([P, BS], F32, tag="i")
                    o_sb = wpool.tile([P, BS], F32, tag="o")
                    g_sb = wpool.tile([P, BS], F32, tag="g")
                    nc.scalar.activation(f_sb[:], fio_ps[0][:], AF.Sigmoid,
                                         bias=bcol(0), scale=FP8_DESCALE)
                    nc.scalar.activation(i_sb[:], fio_ps[1][:], AF.Sigmoid,
                                         bias=bcol(1), scale=FP8_DESCALE)
                    nc.scalar.activation(o_sb[:], fio_ps[2][:], AF.Sigmoid,
                                         bias=bcol(3), scale=FP8_DESCALE)
                    nc.scalar.activation(g_sb[:], g_ps[:], AF.Tanh,
                                         bias=bcol(2))

                    if t == 0:
                        nc.vector.tensor_mul(out=c_sb[:, j, :], in0=i_sb[:],
                                             in1=g_sb[:])
                    else:
                        # in-place: c *= f; g_sb = i*g; c += g_sb
                        nc.vector.tensor_mul(out=c_sb[:, j, :], in0=f_sb[:],
                                             in1=c_sb[:, j, :])
                        nc.vector.tensor_mul(out=g_sb[:], in0=i_sb[:],
                                             in1=g_sb[:])
                        nc.vector.tensor_add(out=c_sb[:, j, :],
                                             in0=c_sb[:, j, :], in1=g_sb[:])
                    th = wpool.tile([P, BS], F32, tag="th")
                    nc.scalar.activation(th[:], c_sb[:, j, :], AF.Tanh)
                    nc.vector.tensor_mul(out=h_sb[:, wbuf, j, :], in0=o_sb[:],
                                         in1=th[:])
                    if t + 1 < T:
                        nc.vector.tensor_scalar_mul(
                            h8_sb[:, wbuf, j, :], h_sb[:, wbuf, j, :],
                            FP8_SCALE)

            # ---- output projection ----
            hbuf = T % 2
            QW = KH * VN // 4  # wout tile loaded in 4 quarters for overlap
            for vt in range(VT):
                vn = min(VN, V - vt * VN)
                wo_sb = wopool.tile([P, KH * VN], BF16, tag="wo")
                for q in range(4):
                    nc.sync.dma_start(out=wo_sb[:, q * QW:(q + 1) * QW],
                                      in_=wo_d.ap()[vt][:, q * QW:(q + 1) * QW])
                for bt in range(NBT):
                    ps = pspool.tile([P, VN], F32, tag="ps")
                    for k in range(KH):
                        nc.tensor.matmul(
                            ps[:, :vn],
                            lhsT=h_sb[:, hbuf, k, bt * P:(bt + 1) * P],
                            rhs=wo_sb[:, k * VN:k * VN + vn],
                            start=(k == 0),
                            stop=(k == KH - 1),
                        )
                    ot = opool.tile([P, VN], F32, tag="ot")
                    nc.vector.tensor_copy(out=ot[:, :vn], in_=ps[:, :vn])
                    # logit writes go out on the ACT HWDGE queue so they
                    # don't contend with the wout reads on the sync queue
                    nc.scalar.dma_start(
                        out=out_d.ap()[bt * P:(bt + 1) * P,
                                       vt * VN:vt * VN + vn],
                        in_=ot[:, :vn])

    nc.compile()
    return nc


def get_nc():
    if "nc" not in _CACHE:
        _CACHE["nc"] = _build()
    return _CACHE["nc"]


def _prep_shared(Emb, WF, WI, WC, WO, bF, bI, bC, bO, Wout):
    emb = np.ascontiguousarray(np.asarray(Emb, dtype=np.float32)).astype(_BF)

    # f/i/o gate weights: fp8 e4m3, pre-scaled by 2^8
    W3 = np.concatenate([np.asarray(WF), np.asarray(WI), np.asarray(WO)],
                        0).astype(np.float32).T          # [1536, 3072]
    wt8 = np.ascontiguousarray(
        W3.reshape(KHX, P, 3 * H).transpose(1, 0, 2))    # [128, 12, 3072]
    wt8 = np.clip(wt8 * FP8_SCALE, -240.0, 240.0).astype(_F8)

    # g gate weights: bf16
    WG = np.asarray(WC).astype(np.float32).T             # [1536, 1024]
    wtg = np.ascontiguousarray(
        WG.reshape(KHX, P, H).transpose(1, 0, 2)).astype(_BF)  # [128, 12, 1024]

    b_all = np.concatenate([np.asarray(bF), np.asarray(bI), np.asarray(bC),
                            np.asarray(bO)], 0).astype(np.float32)  # [4096]
    bias = np.ascontiguousarray(b_all.reshape(4 * H // P, P).T)  # [128, 32]

    Wout = np.asarray(Wout, dtype=np.float32)
    wpad = np.zeros((VPAD, H), np.float32)
    wpad[:V] = Wout
    wo = np.ascontiguousarray(
        wpad.reshape(VT, VN, KH, P).transpose(0, 3, 2, 1).reshape(VT, P, KH * VN)
    ).astype(_BF)  # [63, 128, 4096]
    return emb, wt8, wtg, bias, wo


def kernel(X, Emb, WF, bF, WI, bI, WC, bC, WO, bO, Wout, bout):
    global LAST_RESULTS
    nc = get_nc()

    emb, wt8, wtg, bias, wo = _prep_shared(Emb, WF, WI, WC, WO, bF, bI, bC,
                                           bO, Wout)
    X = np.asarray(X).astype(np.int32)  # [4096, 5]

    in_maps = []
    for c in range(NCORES):
        xs = X[c * BS:(c + 1) * BS]                       # [512, 5]
        idx = np.ascontiguousarray(
            xs.T.reshape(NG, P).T).astype(np.int32)       # [128, 20] t-major
        in_maps.append({"idx": idx, "emb": emb, "wt8": wt8, "wtg": wtg,
                        "bias": bias, "wo": wo})

    res = run_bass_kernel_spmd(nc, in_maps, core_ids=list(range(NCORES)))
    LAST_RESULTS = res

    out = np.concatenate([res.results[c]["out"] for c in range(NCORES)], 0)
    bout = np.asarray(bout, dtype=np.float32)
    if np.any(bout):
        out = out + bout[None, :]
    return out


# revision 10
# speedup vs baseline: 1.4366x; 1.0442x over previous
"""TextLSTM kernel for 8 Trainium2 NeuronCores.

Data-parallel over batch: each of the 8 cores runs the full model on a
512-row batch shard.

Per-core pipeline (feature-major LSTM):
  0. PE warmup: dummy matmuls on a zeroed SBUF tile run before the big
     weight DMAs are allowed to start (pinned via dummy writes).  If
     large-descriptor DMA is active while the PE ramps out of its low
     p-state, the PE clock latches at 2.0 GHz instead of 2.4 GHz for the
     whole execution (+20% on every matmul).  Small-descriptor DMAs (idx
     load, embedding gathers) are latch-safe and start immediately.  A
     second dummy block keeps the PE busy while the gathers land (an idle
     PE + active DMA would re-latch the derate).
  1. Embedding gather: indirect-DMA 2560 rows of Emb (t-major token order)
     into SBUF batch-major, PE-transpose 128x128 blocks into feature-major
     xT[t] tiles, stored as 256*x in bf16 and fp8e4m3.
  2. LSTM recurrence, 5 steps.  All h-recurrent matmuls and the f/i/o
     x-parts run in fp8 DoubleRow mode (2 k-tiles per instruction, 2x PE
     throughput).  fp8 operands carry a 2^8 scale each; the g gate's
     x-part stays bf16 but its operands also carry 2^8 each, so every
     gate PSUM arrives at 2^16 * preact and a single activation descale
     (2^-16) + bias handles all four gates.  fp8 noise on f/i/o is damped
     ~100x by the sigmoids; on g's h-part it contributes <1% output error.
     Cell math fp32 on VectorE; h stored as 256*h in bf16 + fp8
     (double-buffered), c fp32.
  3. Projection: out[512b, 32000v] = h.T @ WoutT streamed per 512-col
     vocab tile (bf16 weights, fp32 PSUM; PSUM carries 2^8 * logits, the
     drain rescales by 2^-8).

Weights are pre-transposed/tiled/cast on the host; biases are all zero per
the problem spec (gate biases are still applied on-device via the activation
bias port; bout is added on host only if nonzero).
"""

import os
import sys

import numpy as np
import ml_dtypes

for _p in ("/opt/trn_rl_repo", "/root/.axon_site/_ro/trn_rl_repo"):
    if os.path.isdir(_p) and _p not in sys.path:
        sys.path.append(_p)

from concourse import bacc, mybir
import concourse.tile as tile
from concourse.bass import IndirectOffsetOnAxis
from concourse.bass_utils import run_bass_kernel_spmd
from concourse.masks import make_identity

P = 128
B, T, E, H, V = 4096, 5, 512, 1024, 32000
NCORES = 8
BS = B // NCORES          # 512 batch rows per core
NTOK = BS * T             # 2560 gathered tokens per core
NG = NTOK // P            # 20 gather tiles of 128 tokens
KH = H // P               # 8 k-tiles over h
KE = E // P               # 4 k-tiles over x
KHX = KH + KE             # 12 k-tiles over [h; x]
NJ = H // P               # 8 hidden-dim tiles
VN = 512                  # vocab tile width
VT = (V + VN - 1) // VN   # 63 vocab tiles (last one 256 wide)
VPAD = VT * VN            # 32256
NBT = BS // P             # 4 batch tiles

F32 = mybir.dt.float32
BF16 = mybir.dt.bfloat16
FP8 = mybir.dt.float8e4
I32 = mybir.dt.int32
AF = mybir.ActivationFunctionType
DR = mybir.MatmulPerfMode.DoubleRow
MUL = mybir.AluOpType.mult
MAX = mybir.AluOpType.max

_BF = ml_dtypes.bfloat16
_F8 = ml_dtypes.float8_e4m3

FP8_SCALE = 256.0         # 2^8 pre-scale on each gate-matmul operand
GATE_DESCALE = 1.0 / (FP8_SCALE * FP8_SCALE)
PROJ_DESCALE = 1.0 / FP8_SCALE

WARM1 = 28                # PE ramp matmuls before big-DMA release
WARM2 = 48                # PE filler matmuls while gathers/weights land

_CACHE = {}
LAST_RESULTS = None


def _build():
    nc = bacc.Bacc("TRN2", target_bir_lowering=False, debug=False,
                   num_devices=NCORES)

    idx_d = nc.dram_tensor("idx", [P, NG], I32, kind="ExternalInput")
    emb_d = nc.dram_tensor("emb", [V, E], BF16, kind="ExternalInput")
    wt8_d = nc.dram_tensor("wt8", [P, KHX, 4 * H], FP8, kind="ExternalInput")
    wtg_d = nc.dram_tensor("wtg", [P, KE, H], BF16, kind="ExternalInput")
    bias_d = nc.dram_tensor("bias", [P, 4 * H // P], F32, kind="ExternalInput")
    wo_d = nc.dram_tensor("wo", [VT, P, KH * VN], BF16, kind="ExternalInput")
    out_d = nc.dram_tensor("out", [BS, V], F32, kind="ExternalOutput")

    with tile.TileContext(nc) as tc:
        with (
            tc.tile_pool(name="const", bufs=1) as cpool,
            tc.tile_pool(name="gather", bufs=6) as gpool,
            tc.tile_pool(name="work", bufs=2) as wpool,
            tc.tile_pool(name="woutp", bufs=3) as wopool,
            tc.tile_pool(name="outp", bufs=4) as opool,
            tc.tile_pool(name="psum", bufs=8, space="PSUM") as pspool,
        ):
            scratch = cpool.tile([P, VN], BF16, tag="scratch")
            nc.vector.memset(scratch[:], 0)
            ident = cpool.tile([P, P], BF16, tag="ident")
            make_identity(nc, ident[:])

            # persistent SBUF state
            wt8_sb = cpool.tile([P, KHX, 4 * H], FP8, tag="wt8")
            wtg_sb = cpool.tile([P, KE, H], BF16, tag="wtg")
            bias_sb = cpool.tile([P, 4 * H // P], F32, tag="bias")
            h_sb = cpool.tile([P, 2, KH, BS], BF16, tag="h")
            h8_sb = cpool.tile([P, 2, KH, BS], FP8, tag="h8")
            c_sb = cpool.tile([P, NJ, BS], F32, tag="c")
            xt_sb = cpool.tile([P, T, KE, BS], BF16, tag="xt")
            x8_sb = cpool.tile([P, T, KE, BS], FP8, tag="x8")
            idx_sb = cpool.tile([P, NG], I32, tag="idx")

            # small-descriptor DMAs (idx load + embedding gathers) are
            # latch-safe and start immediately; only the big weight-chunk
            # loads must wait for the PE clock ramp.
            nc.sync.dma_start(out=idx_sb[:], in_=idx_d.ap())
            nc.sync.dma_start(out=bias_sb[:], in_=bias_d.ap())
            xgs = []
            for g in range(NG):
                xg = gpool.tile([P, E], BF16, tag="xg")
                nc.gpsimd.indirect_dma_start(
                    out=xg[:],
                    out_offset=None,
                    in_=emb_d.ap(),
                    in_offset=IndirectOffsetOnAxis(ap=idx_sb[:, g:g + 1], axis=0),
                )
                xgs.append(xg)

            # ---- PE warmup: ramp the clock before the big DMAs start ----
            # (dummy matmuls on the zeroed scratch tile, which needs no DMA)
            ps_w = pspool.tile([P, P], F32, tag="ps", name="ps_warm")
            for _ in range(WARM1):
                nc.tensor.matmul(ps_w[:], lhsT=scratch[:, :P],
                                 rhs=scratch[:, :P], start=True, stop=True)

            # pin the weight loads behind the warmup: a dummy vector write
            # into each DMA destination creates a WAW dependency.
            def pin(dst_ap):
                nc.vector.tensor_copy(out=dst_ap, in_=ps_w[:, :1])

            # x-part weights first: they gate the t=0 matmuls, the h-part
            # loads overlap with t=0 compute.
            for kt in range(KE):
                pin(wtg_sb[:, kt, :2].bitcast(F32)[:, :1])
                nc.sync.dma_start(out=wtg_sb[:, kt, :], in_=wtg_d.ap()[:, kt, :])
            for kt in list(range(KH, KHX)) + list(range(KH)):
                pin(wt8_sb[:, kt, :4].bitcast(F32)[:, :1])
                nc.sync.dma_start(out=wt8_sb[:, kt, :], in_=wt8_d.ap()[:, kt, :])

            # keep the PE busy while the gathers/x-weights stream in, so it
            # never idles with DMA active (idle + active DMA re-derates).
            ps_w2 = pspool.tile([P, P], F32, tag="ps", name="ps_warm2")
            for _ in range(WARM2):
                nc.tensor.matmul(ps_w2[:], lhsT=scratch[:, :P],
                                 rhs=scratch[:, :P], start=True, stop=True)

            # PE-transpose one step's gather tiles into feature-major
            # 256*x tiles (bf16 for the g-gate, fp8 for f/i/o)
            def emit_transposes(tt):
                for bb in range(NBT):
                    xg = xgs[tt * NBT + bb]
                    for e in range(KE):
                        ps_tr = pspool.tile([P, P], BF16, tag="ps",
                                            name="ps_tr")
                        nc.tensor.transpose(
                            ps_tr[:], xg[:, e * P:(e + 1) * P], ident[:])
                        nc.vector.tensor_scalar_mul(
                            xt_sb[:, tt, e, bb * P:(bb + 1) * P],
                            ps_tr[:], FP8_SCALE)
                        nc.vector.tensor_scalar_mul(
                            x8_sb[:, tt, e, bb * P:(bb + 1) * P],
                            ps_tr[:], FP8_SCALE)

            # ---- LSTM recurrence ----
            emit_transposes(0)
            for t in range(T):
                rbuf, wbuf = t % 2, (t + 1) % 2
                # x-part k-pairs first: their rhs (x8/xt) is ready
                # immediately, so PE enters the step while the tail of the
                # previous step's h writes is still in flight.
                if t > 0:
                    prs = [KH, KH + 2, 0, 2, 4, 6]
                    prs_h = [0, 2, 4, 6]
                else:
                    prs = [KH, KH + 2]
                    prs_h = []

                for j in range(NJ):
                    # next step's transposes go mid-stream, where PSUM slots
                    # are freshly recycled — not at the step boundary where
                    # they'd contend with the previous step's gate drains
                    if j == 1 and t + 1 < T:
                        emit_transposes(t + 1)

                    # f, i, o gates: fp8 DoubleRow (2 k-tiles / instr)
                    fio_ps = []
                    for gi in range(3):
                        ps = pspool.tile([P, VN], F32, tag="ps")
                        col = gi * H + j * P
                        for n, k0 in enumerate(prs):
                            if k0 >= KH:
                                rhs = x8_sb[:, t, k0 - KH:k0 - KH + 2, :]
                            else:
                                rhs = h8_sb[:, rbuf, k0:k0 + 2, :]
                            nc.tensor.matmul(
                                ps[:],
                                lhsT=wt8_sb[:, k0:k0 + 2, col:col + P],
                                rhs=rhs,
                                start=(n == 0),
                                stop=(n == len(prs) - 1),
                                perf_mode=DR,
                            )
                        fio_ps.append(ps)

                    # g gate: bf16 x-part (256-scaled operands) + fp8
                    # DoubleRow h-part, accumulated in one PSUM at 2^16
                    g_ps = pspool.tile([P, VN], F32, tag="ps")
                    colg = 3 * H + j * P
                    for n, e in enumerate(range(KE)):
                        nc.tensor.matmul(
                            g_ps[:],
                            lhsT=wtg_sb[:, e, j * P:j * P + P],
                            rhs=xt_sb[:, t, e, :],
                            start=(n == 0),
                            stop=(t == 0 and n == KE - 1),
                        )
                    for n, k0 in enumerate(prs_h):
                        nc.tensor.matmul(
                            g_ps[:],
                            lhsT=wt8_sb[:, k0:k0 + 2, colg:colg + P],
                            rhs=h8_sb[:, rbuf, k0:k0 + 2, :],
                            start=False,
                            stop=(n == len(prs_h) - 1),
                            perf_mode=DR,
                        )

                    bcol = lambda gi: bias_sb[:, gi * NJ + j:gi * NJ + j + 1]
                    f_sb = wpool.tile([P, BS], F32, tag="f")
                    i_sb = wpool.tile([P, BS], F32, tag="i")
                    o_sb = wpool.tile([P, BS], F32, tag="o")
                    g_sb = wpool.tile([P, BS], F32, tag="g")
                    nc.scalar.activation(f_sb[:], fio_ps[0][:], AF.Sigmoid,
                                         bias=bcol(0), scale=GATE_DESCALE)
                    nc.scalar.activation(i_sb[:], fio_ps[1][:], AF.Sigmoid,
                                         bias=bcol(1), scale=GATE_DESCALE)
                    nc.scalar.activation(o_sb[:], fio_ps[2][:], AF.Sigmoid,
                                         bias=bcol(3), scale=GATE_DESCALE)
                    nc.scalar.activation(g_sb[:], g_ps[:], AF.Tanh,
                                         bias=bcol(2), scale=GATE_DESCALE)

                    if t == 0:
                        nc.vector.tensor_mul(out=c_sb[:, j, :], in0=i_sb[:],
                                             in1=g_sb[:])
                    else:
                        # in-place: c *= f; g_sb = i*g; c += g_sb
                        nc.vector.tensor_mul(out=c_sb[:, j, :], in0=f_sb[:],
                                             in1=c_sb[:, j, :])
                        nc.vector.tensor_mul(out=g_sb[:], in0=i_sb[:],
                                             in1=g_sb[:])
                        nc.vector.tensor_add(out=c_sb[:, j, :],
                                             in0=c_sb[:, j, :], in1=g_sb[:])
                    th = wpool.tile([P, BS], F32, tag="th")
                    nc.scalar.activation(th[:], c_sb[:, j, :], AF.Tanh)
                    # h_sb = o * tanh(c) (bf16, unscaled — only the
                    # projection reads it); h8 = fp8(256 * h)
                    nc.vector.tensor_mul(out=h_sb[:, wbuf, j, :], in0=o_sb[:],
                                         in1=th[:])
                    if t + 1 < T:
                        nc.vector.tensor_scalar_mul(h8_sb[:, wbuf, j, :],
                                                    h_sb[:, wbuf, j, :],
                                                    FP8_SCALE)

            # ---- output projection ----
            hbuf = T % 2
            QW = KH * VN // 4  # wout tile loaded in 4 quarters for overlap
            for vt in range(VT):
                vn = min(VN, V - vt * VN)
                wo_sb = wopool.tile([P, KH * VN], BF16, tag="wo")
                for q in range(4):
                    nc.sync.dma_start(out=wo_sb[:, q * QW:(q + 1) * QW],
                                      in_=wo_d.ap()[vt][:, q * QW:(q + 1) * QW])
                for bt in range(NBT):
                    ps = pspool.tile([P, VN], F32, tag="ps")
                    for k in range(KH):
                        nc.tensor.matmul(
                            ps[:, :vn],
                            lhsT=h_sb[:, hbuf, k, bt * P:(bt + 1) * P],
                            rhs=wo_sb[:, k * VN:k * VN + vn],
                            start=(k == 0),
                            stop=(k == KH - 1),
                        )
                    ot = opool.tile([P, VN], F32, tag="ot")
                    nc.vector.tensor_copy(out=ot[:, :vn], in_=ps[:, :vn])
                    # logit writes go out on the ACT HWDGE queue so they
                    # don't contend with the wout reads on the sync queue
                    nc.scalar.dma_start(
                        out=out_d.ap()[bt * P:(bt + 1) * P,
                                       vt * VN:vt * VN + vn],
                        in_=ot[:, :vn])

    nc.compile()
    return nc


def get_nc():
    if "nc" not in _CACHE:
        _CACHE["nc"] = _build()
    return _CACHE["nc"]


def _prep_shared(Emb, WF, WI, WC, WO, bF, bI, bC, bO, Wout):
    emb = np.ascontiguousarray(np.asarray(Emb, dtype=np.float32)).astype(_BF)

    # all gate weights: fp8 e4m3, pre-scaled by 2^8 (f,i,o + c for h-part)
    W4 = np.concatenate([np.asarray(WF), np.asarray(WI), np.asarray(WO),
                         np.asarray(WC)], 0).astype(np.float32).T  # [1536,4096]
    wt8 = np.ascontiguousarray(
        W4.reshape(KHX, P, 4 * H).transpose(1, 0, 2))    # [128, 12, 4096]
    wt8 = np.clip(wt8 * FP8_SCALE, -240.0, 240.0).astype(_F8)

    # g gate x-part weights: bf16, pre-scaled by 2^8
    WG = np.asarray(WC).astype(np.float32).T[H:]         # [512, 1024] x rows
    wtg = np.ascontiguousarray(
        (WG * FP8_SCALE).reshape(KE, P, H).transpose(1, 0, 2)).astype(_BF)

    b_all = np.concatenate([np.asarray(bF), np.asarray(bI), np.asarray(bC),
                            np.asarray(bO)], 0).astype(np.float32)  # [4096]
    bias = np.ascontiguousarray(b_all.reshape(4 * H // P, P).T)  # [128, 32]

    Wout = np.asarray(Wout, dtype=np.float32)
    wpad = np.zeros((VPAD, H), np.float32)
    wpad[:V] = Wout
    wo = np.ascontiguousarray(
        wpad.reshape(VT, VN, KH, P).transpose(0, 3, 2, 1).reshape(VT, P, KH * VN)
    ).astype(_BF)  # [63, 128, 4096]
    return emb, wt8, wtg, bias, wo


def kernel(X, Emb, WF, bF, WI, bI, WC, bC, WO, bO, Wout, bout):
    global LAST_RESULTS
    nc = get_nc()

    emb, wt8, wtg, bias, wo = _prep_shared(Emb, WF, WI, WC, WO, bF, bI, bC,
                                           bO, Wout)
    X = np.asarray(X).astype(np.int32)  # [4096, 5]

    in_maps = []
    for c in range(NCORES):
        xs = X[c * BS:(c + 1) * BS]                       # [512, 5]
        idx = np.ascontiguousarray(
            xs.T.reshape(NG, P).T).astype(np.int32)       # [128, 20] t-major
        in_maps.append({"idx": idx, "emb": emb, "wt8": wt8, "wtg": wtg,
                        "bias": bias, "wo": wo})

    res = run_bass_kernel_spmd(nc, in_maps, core_ids=list(range(NCORES)))
    LAST_RESULTS = res

    out = np.concatenate([res.results[c]["out"] for c in range(NCORES)], 0)
    bout = np.asarray(bout, dtype=np.float32)
    if np.any(bout):
        out = out + bout[None, :]
    return out
